# revision 31
# baseline (speedup 1.0000x reference)
"""Multi-head attention (B=2, S=2048, D=1024, H=16) on 8 trn2 NeuronCores.

Sharding: core c handles batch b = c//4 and head-group g = c%4 (4 heads).
Megatron-style: Wq/Wk/Wv column-split, Wo row-split; host sums the 4 partial
outputs per batch and adds bo.

v2 schedule: the scalar engine (exp over S^2 scores, 128 x [128,1024]
instructions ~142us) is the roofline; everything else is arranged so ACT
never starves:
  - inputs DMA'd in need-order chunks (i-block-major xq, key-quad-major xk,
    j-half-major xv) across the Sync/Pool/Vector queues while the first
    projection groups run; a dummy exp preloads the ACT table during the fill
  - per-slot worklists spread qk-projection (2-matmul chunks), v-projection
    (arrival-ordered, w0 late slots + w1), the output projection (w5-w7),
    and window-finish chains across all 8 windows
  - AV is drained lazily from a pending queue: oldest window first, only
    v-ready j's (any j order within a window's psum accumulation group),
    finish fires automatically on the 16th block
  - softmax denominators ride the AV matmul as a 65th ones-column (row 64
    of the [65, IB] psum accumulator)
The 1/sqrt(64) score scale is folded into Wq/bq on the host.
"""

import numpy as np
import ml_dtypes
from contextlib import ExitStack

import concourse.bass as bass
import concourse.tile as tile
from concourse import bacc, mybir
from concourse.bass_utils import run_bass_kernel_spmd

F32 = mybir.dt.float32
BF16 = mybir.dt.bfloat16
AF = mybir.ActivationFunctionType

D_MODEL = 1024
NUM_HEADS = 16
DK = 64
B = 2
S = 2048
NG = 4  # head groups = cores per batch
HPG = 4  # heads per group
CG = HPG * DK  # 256 channels per group
KIN = D_MODEL + 1  # wv rows: 1024 + bias row
VW = HPG * (DK + 1)  # 260: per-head [v_h | ones]
IB = 512  # i-block (query) width
NI = S // IB
NJ = S // 128
NK = D_MODEL // 128
NQ = 4  # key quads

_CACHE = {}


def build_program():
    nc = bacc.Bacc("TRN2", target_bir_lowering=False, debug=False, num_devices=8)
    # x tensors are pre-laid-out host-side in SBUF shape [128, 4*NK*IB]:
    # chunk c (i-block / key-quad / token-quarter) at c*NK*IB, k-tile at +k*IB.
    # Every DMA below is a contiguous 2D copy (128 descriptors).
    xq_d = nc.dram_tensor("xq", [128, NI * NK * IB], BF16, kind="ExternalInput")
    xk_d = nc.dram_tensor("xk", [128, NQ * NK * IB], BF16, kind="ExternalInput")
    xv_d = nc.dram_tensor("xv", [128, NQ * NK * IB], BF16, kind="ExternalInput")
    wq_d = nc.dram_tensor("wq", [128, NK * CG], BF16, kind="ExternalInput")
    wk_d = nc.dram_tensor("wk", [128, NK * CG], BF16, kind="ExternalInput")
    bqk_d = nc.dram_tensor("bqk", [128, 4], F32, kind="ExternalInput")
    wv_d = nc.dram_tensor("wv", [128, NK * VW + VW], BF16, kind="ExternalInput")
    wo_d = nc.dram_tensor("wo", [CG, D_MODEL], BF16, kind="ExternalInput")
    out_d = nc.dram_tensor("out", [S, D_MODEL], F32, kind="ExternalOutput")

    windows = [(pr, i) for pr in range(2) for i in range(NI)]

    with tile.TileContext(nc) as tc, ExitStack() as ctx:
        wpool = ctx.enter_context(tc.tile_pool(name="wpool", bufs=1))
        xpool = ctx.enter_context(tc.tile_pool(name="xpool", bufs=1))
        qkvpool = ctx.enter_context(tc.tile_pool(name="qkv", bufs=1))
        attnpool = ctx.enter_context(tc.tile_pool(name="attn", bufs=1))
        spsum = ctx.enter_context(tc.tile_pool(name="spsum", bufs=2, space="PSUM"))
        apsum = ctx.enter_context(tc.tile_pool(name="apsum", bufs=1, space="PSUM"))
        ph1psum = ctx.enter_context(tc.tile_pool(name="ph1psum", bufs=1, space="PSUM"))
        epool = ctx.enter_context(tc.tile_pool(name="epool", bufs=17))
        ppool = ctx.enter_context(tc.tile_pool(name="ppool", bufs=3))
        rpool = ctx.enter_context(tc.tile_pool(name="rpool", bufs=2))
        obounce = ctx.enter_context(tc.tile_pool(name="obounce", bufs=2))

        # ---- SBUF input layouts ----
        # xq: i-block-major; xk: key-quad-major; xv: token-quarter-major
        # (chunk c at c*NK*IB, k-tile k at +k*IB)
        xq_sb = xpool.tile([128, NI * NK * IB], BF16)
        xk_sb = xpool.tile([128, NQ * NK * IB], BF16)
        xv_sb = xpool.tile([128, NQ * NK * IB], BF16)
        wq_sb = wpool.tile([128, NK * CG], BF16)
        wk_sb = wpool.tile([128, NK * CG], BF16)
        wv_sb = wpool.tile([128, NK * VW], BF16)
        bqk_sb = wpool.tile([128, 4], F32)  # cols: [q_cb0, k_cb0, q_cb1, k_cb1]
        wvb = wpool.tile([1, VW], BF16)  # v bias row (incl. ones-col entries)
        xvon = wpool.tile([1, 128], BF16)  # ones row (memset, no DMA)
        ones128 = wpool.tile([128, 1], BF16)  # ones col for den matmuls
        wo_sb = [wpool.tile([128, D_MODEL], BF16, name=f"wo{t}") for t in range(2)]
        dum = wpool.tile([1, 8], F32)
        dume = wpool.tile([1, 8], BF16)

        # ---- prologue: prioritized input DMA + act-table preload ----
        nc.gpsimd.memset(xvon[:], 1.0)
        nc.gpsimd.memset(ones128[:], 1.0)
        nc.gpsimd.memset(dum[:], 0.0)

        def dma_x(eng, sb, src, c):
            eng.dma_start(
                sb[:, c * NK * IB : (c + 1) * NK * IB],
                src.ap()[:, c * NK * IB : (c + 1) * NK * IB],
            )

        # Two HWDGE queues only (the gpsimd SWDGE queue is far slower and its
        # all-run-long trickle writes contend with ACT's SBUF writes).
        # Priority: weights -> xq-i0/xk (w0 scores path) -> xq-i1 -> xv
        # quarters -> xq-i2/i3 (needed only from w2 on) -> wo.
        nc.sync.dma_start(bqk_sb[:], bqk_d.ap())
        nc.sync.dma_start(wq_sb[:], wq_d.ap())
        dma_x(nc.sync, xq_sb, xq_d, 0)  # i0
        dma_x(nc.sync, xk_sb, xk_d, 1)  # quad 1
        dma_x(nc.sync, xk_sb, xk_d, 3)  # quad 3
        dma_x(nc.sync, xv_sb, xv_d, 0)
        dma_x(nc.sync, xv_sb, xv_d, 2)
        dma_x(nc.sync, xq_sb, xq_d, 2)  # i2
        dma_x(nc.sync, xq_sb, xq_d, 3)  # i3
        # Scalar queue: all issues happen at t=0 while ACT is otherwise idle
        # (first exp isn't ready until ~15us)
        nc.scalar.dma_start(wk_sb[:], wk_d.ap())
        dma_x(nc.scalar, xk_sb, xk_d, 0)  # quad 0
        dma_x(nc.scalar, xk_sb, xk_d, 2)  # quad 2
        dma_x(nc.scalar, xq_sb, xq_d, 1)  # i1
        nc.scalar.dma_start(wvb[:], wv_d.ap()[0:1, NK * VW : NK * VW + VW])
        nc.scalar.dma_start(wv_sb[:], wv_d.ap()[:, : NK * VW])
        dma_x(nc.scalar, xv_sb, xv_d, 1)
        dma_x(nc.scalar, xv_sb, xv_d, 3)
        for t in range(2):
            nc.scalar.dma_start(wo_sb[t][:], wo_d.ap()[t * 128 : (t + 1) * 128, :])
        nc.scalar.activation(dume[:], dum[:], AF.Exp)  # preload the exp table set

        # ---- persistent compute tiles ----
        qTt = [[qkvpool.tile([128, IB], BF16, name=f"qT{cb}_{i}") for i in range(NI)]
               for cb in range(2)]
        kTt = [[qkvpool.tile([128, IB], BF16, name=f"kT{cb}_{q}") for q in range(NQ)]
               for cb in range(2)]
        vt = [qkvpool.tile([128, VW], BF16, name=f"v{j}") for j in range(NJ)]
        attt = [[attnpool.tile([128, IB], BF16, name=f"att{t}_{i}") for i in range(NI)]
                for t in range(2)]

        # ---- work-unit emitters ----
        qk_state = {}

        def qk_chunk(cb, tsel, i, c):
            """chunk c (0..3) of projection group -> qTt/kTt[cb][i]; 2 mms each."""
            xt, xoff, w_sb, dst, bcol = (
                (xq_sb, i * NK * IB, wq_sb, qTt, 0)
                if tsel == 0
                else (xk_sb, i * NK * IB, wk_sb, kTt, 1)
            )
            key = (cb, tsel, i)
            if c == 0:
                qk_state[key] = ph1psum.tile([128, IB], F32, name="pqk")
            pq = qk_state[key]
            for k in (2 * c, 2 * c + 1):
                nc.tensor.matmul(
                    pq[:],
                    w_sb[:, k * CG + cb * 128 : k * CG + (cb + 1) * 128],
                    xt[:, xoff + k * IB : xoff + (k + 1) * IB],
                    start=(k == 0),
                    stop=(k == NK - 1),
                )
            if c == 3:
                nc.vector.tensor_scalar_add(
                    dst[cb][i][:], pq[:], bqk_sb[:, 2 * cb + bcol : 2 * cb + bcol + 1]
                )
                del qk_state[key]

        vg_done = set()

        def v_group(j):
            """v[j, e] = sum_k xv[k, j] wv[k, e] (+bias/ones via the 1025th row)"""
            vq, jj = j // 4, j % 4
            pv = ph1psum.tile([128, IB], F32, name="p0")[:, :VW]
            for k in range(NK):
                nc.tensor.matmul(
                    pv[:],
                    xv_sb[:, vq * NK * IB + k * IB + jj * 128 :
                          vq * NK * IB + k * IB + (jj + 1) * 128],
                    wv_sb[:, k * VW : (k + 1) * VW],
                    start=(k == 0),
                    stop=False,
                )
            nc.tensor.matmul(pv[:], xvon[:], wvb[:], start=False, stop=True)
            nc.vector.tensor_copy(vt[j][:], pv[:])
            vg_done.add(j)

        # AV: lagged any-j-order accumulation per window.
        # Windows < NPACK (the PE-crowded ramp-up) use column-packed AV: both
        # heads' [128k, 64] v tiles in PE col-groups (0,0)/(0,64), streaming
        # their ee halves concurrently -> half the PE time. Softmax sums come
        # from DVE pair-added ee tiles pushed through tiny ones-matmuls into
        # psum rows 0/32 of the second bank. Windows >= NPACK use the ones-
        # column AV ([65, IB] accumulators, sums in row 64).
        NPACK = 3
        pending = []  # (w, j, ee)
        aa_of = {}
        av_count = {}
        den_state = {}

        def emit_av(w, j, ee):
            pr, i = windows[w]
            if w not in aa_of:
                a0 = apsum.tile([128, IB], F32, name="a0")
                a1 = apsum.tile([DK + 1, IB], F32, name="a1")
                aa_of[w] = (a0, a1)
                av_count[w] = 0
            a0, a1 = aa_of[w]
            first = av_count[w] == 0
            last = av_count[w] == NJ - 1
            if w < NPACK:
                for u in range(2):
                    h = 2 * pr + u
                    nc.tensor.matmul(
                        a0[u * DK : (u + 1) * DK, :],
                        vt[j][:, h * (DK + 1) : h * (DK + 1) + DK],
                        ee[:, u * IB : (u + 1) * IB],
                        start=first,
                        stop=last,
                        tile_position=(0, u * DK),
                        skip_group_check=True,
                    )
            else:
                for u, dst in ((0, a0[0 : DK + 1, :]), (1, a1[:])):
                    h = 2 * pr + u
                    nc.tensor.matmul(
                        dst,
                        vt[j][:, h * (DK + 1) : (h + 1) * (DK + 1)],
                        ee[:, u * IB : (u + 1) * IB],
                        start=first,
                        stop=last,
                        skip_group_check=True,
                    )
            av_count[w] += 1
            if w < NPACK:
                flush_dens(w)

        def flush_dens(w):
            """emit queued den ones-matmuls once aa_of[w] exists (after the
            first AV of w, which itself waits for finish(w-1) -> no PE-queue
            deadlock on the psum WAR)"""
            st = den_state.get(w)
            if st is None or w not in aa_of:
                return
            _, a1 = aa_of[w]
            while st["adds"]:
                pp = st["adds"].pop(0)
                first = st["pairs"] == 0
                last = st["pairs"] == NJ // 2 - 1
                for u in range(2):
                    nc.tensor.matmul(
                        a1[32 * u : 32 * u + 1, :],
                        ones128[:],
                        pp[:, u * IB : (u + 1) * IB],
                        start=first,
                        stop=last,
                        tile_position=(0, 32 * u),
                        skip_group_check=True,
                    )
                st["pairs"] += 1

        def finish_window(w):
            """normalize A^T by the softmax sums -> attt[pr][i]"""
            pr, i = windows[w]
            a0, a1 = aa_of.pop(w)
            dq = nc.scalar if w == 7 else nc.sync  # tail chain off the busy queue
            if w < NPACK:
                asb = rpool.tile([128, IB], F32, name="asb")
                nc.vector.tensor_copy(asb[:], a0[:])
                sd = rpool.tile([33, IB], F32, name="sd")
                nc.vector.tensor_copy(sd[:], a1[0:33, :])
                r8 = rpool.tile([128, 8], F32, name="r8")
                nc.sync.dma_start(r8[:, 0:4], sd[0:1, :])
                nc.sync.dma_start(r8[:, 4:8], sd[32:33, :])
                r8b = rpool.tile([128, 8], F32, name="r8b")
                nc.vector.reciprocal(r8b[:], r8[:])
                rb = rpool.tile([128, IB], F32, name="rb")
                for u in range(2):
                    r0 = rpool.tile([1, IB], F32, name="r0")
                    nc.sync.dma_start(r0[:], r8b[:, 4 * u : 4 * (u + 1)])
                    nc.gpsimd.partition_broadcast(rb[u * DK : (u + 1) * DK, :], r0[:])
                nc.vector.tensor_mul(attt[pr][i][:], asb[:], rb[:])
                return
            for u, aau in ((0, a0[0 : DK + 1, :]), (1, a1[:])):
                asb = rpool.tile([128, IB], F32, name="asb")
                nc.vector.tensor_copy(asb[0 : DK + 1, :], aau)
                r4 = rpool.tile([128, 4], F32, name="r4")
                dq.dma_start(r4[:], asb[DK : DK + 1, :])
                r4b = rpool.tile([128, 4], F32, name="r4b")
                nc.vector.reciprocal(r4b[:], r4[:])
                r0 = rpool.tile([1, IB], F32, name="r0")
                dq.dma_start(r0[:], r4b[:])
                rb = rpool.tile([128, IB], F32, name="rb")
                nc.gpsimd.partition_broadcast(rb[0:DK, :], r0[:])
                po = DK * u
                if po == 0:
                    nc.vector.tensor_mul(attt[pr][i][0:DK, :], asb[0:DK, :], rb[0:DK, :])
                else:
                    nrm = rpool.tile([DK, IB], BF16, name="nrm")
                    nc.vector.tensor_mul(nrm[:], asb[0:DK, :], rb[0:DK, :])
                    dq.dma_start(attt[pr][i][po : po + DK, :], nrm[:])

        def av_drain(cur_w, cap, target):
            done = 0
            while done < cap and pending:
                pw0 = pending[0][0]
                if pw0 == cur_w and len(pending) <= target:
                    break
                idx = next(
                    (n for n, (pw, pj, _) in enumerate(pending)
                     if pw == pw0 and pj in vg_done),
                    None,
                )
                if idx is None:
                    break
                pw, pj, pee = pending.pop(idx)
                emit_av(pw, pj, pee)
                done += 1
                if av_count[pw] == NJ:
                    finish_window(pw)
                    break  # give the asb copies a slot before the next window's AV

        def force_drain_through(tw):
            """guarantee finish_window(w<=tw) is emitted before returning"""
            while pending and (pending[0][0] <= tw or tw in aa_of):
                pw0 = pending[0][0]
                if pw0 > tw:
                    break
                idx = next(
                    (n for n, (pw, pj, _) in enumerate(pending)
                     if pw == pw0 and pj in vg_done),
                    None,
                )
                if idx is None:
                    break
                pw, pj, pee = pending.pop(idx)
                emit_av(pw, pj, pee)
                if av_count[pw] == NJ:
                    finish_window(pw)

        def ph3_unit(i, ibl):
            """output projection for one 128-token block; per-mh [128,512] store.
            mh0 uses the p0 bank, mh1 the pqk bank (idle once projections are
            done) so the 4 matmuls run back-to-back and copies overlap."""
            force_drain_through(4 + i)
            for mh, pname in ((0, "p0"), (1, "pqk")):
                po_t = ph1psum.tile([128, IB], F32, name=pname)
                for t in range(2):
                    nc.tensor.matmul(
                        po_t[:],
                        attt[t][i][:, ibl * 128 : (ibl + 1) * 128],
                        wo_sb[t][:, mh * IB : (mh + 1) * IB],
                        start=(t == 0),
                        stop=(t == 1),
                    )
                ob = obounce.tile([128, IB], F32, name="ob")
                nc.vector.tensor_copy(ob[:], po_t[:])
                nc.sync.dma_start(
                    out_d.ap()[(i * 4 + ibl) * 128 : (i * 4 + ibl + 1) * 128,
                               mh * IB : (mh + 1) * IB],
                    ob[:],
                )

        # ---- slot schedule ----
        sched = [dict() for _ in range(8)]

        def put(w, j, fn):
            sched[w].setdefault(j, []).append(fn)

        def put_group(w, j0, cb, tsel, i):
            for c in range(4):
                put(w, j0 + c, lambda cb=cb, t=tsel, i=i, c=c: qk_chunk(cb, t, i, c))

        # qk-projection spread (deadlines: qT[pr][i] before window (pr,i);
        # kT[cb][q] before the scores quad that reads it)
        put_group(0, 3, 0, 1, 2)   # KT(0,2) by w0 slot 8
        put_group(0, 8, 0, 1, 3)   # KT(0,3) by w0 slot 12
        put_group(0, 11, 0, 0, 1)  # QT(0,1) by w1
        put_group(1, 12, 0, 0, 2)  # QT(0,2) by w2 (xq-i2 lands mid-w1)
        put_group(2, 2, 0, 0, 3)
        put_group(2, 6, 1, 1, 0)
        put_group(2, 10, 1, 1, 1)
        put_group(3, 1, 1, 1, 2)
        put_group(3, 5, 1, 1, 3)
        put_group(3, 9, 1, 0, 0)
        put_group(4, 1, 1, 0, 1)
        put_group(4, 8, 1, 0, 2)
        put_group(5, 1, 1, 0, 3)
        # v-projection, arrival-ordered (xv quarters interleave both queues)
        for n, (w, j) in enumerate(
            [(0, 14), (0, 15)]
            + [(1, s) for s in range(12)]
            + [(2, 0), (2, 1)]
        ):
            put(w, j, lambda n=n: v_group(n))
        # output projection: i ready after finish(window (1,i)) fires in w5+i
        for i, w in ((0, 5), (1, 6), (2, 7)):
            for n in range(4):
                put(w, 7 + 2 * n, lambda i=i, n=n: ph3_unit(i, n))

        # AV cadence: (cap, target) per window
        av_plan = [(2, 9), (3, 7), (3, 5), (3, 4), (3, 3), (3, 2), (3, 2), (3, 2)]

        # ---- main loop ----
        _sid, _ = nc.enter_named_scope("steady", False)
        # prologue compute (overlaps the DMA fill)
        for c in range(4):
            qk_chunk(0, 0, 0, c)  # QT(0,0)
        for c in range(4):
            qk_chunk(0, 1, 0, c)  # KT(0,0)
        for c in range(4):
            qk_chunk(0, 1, 1, c)  # KT(0,1)

        for w, (pr, i) in enumerate(windows):
            cap, target = av_plan[w]
            for j in range(NJ):
                ss = spsum.tile([128, 2 * IB], F32, name="ss")
                for u in range(2):
                    nc.tensor.matmul(
                        ss[:, u * IB : (u + 1) * IB],
                        kTt[pr][j // 4][u * DK : (u + 1) * DK,
                                        (j % 4) * 128 : (j % 4 + 1) * 128],
                        qTt[pr][i][u * DK : (u + 1) * DK, :],
                        start=True,
                        stop=True,
                        tile_position=(u * DK, 0),
                    )
                ee = epool.tile([128, 2 * IB], BF16, name="ee")
                nc.scalar.activation(ee[:], ss[:], AF.Exp)
                pending.append((w, j, ee))
                if w < NPACK:
                    st = den_state.setdefault(
                        w, {"eprev": None, "adds": [], "pairs": 0}
                    )
                    if j % 2 == 0:
                        st["eprev"] = ee
                    else:
                        pp = ppool.tile([128, 2 * IB], BF16, name="pp")
                        nc.vector.tensor_add(pp[:], st["eprev"][:], ee[:])
                        st["adds"].append(pp)
                for fn in sched[w].get(j, []):
                    fn()
                av_drain(w, cap, target)

        # ---- tail ----
        while pending:
            av_drain(-1, 99, 0)
        for n in range(4):
            ph3_unit(3, n)
        nc.leave_named_scope("steady", _sid, False)

    nc.compile()
    return nc


def _sbufify_x(xT):
    """[1024 d, 2048 s] -> [128, 4*NK*IB] chunk-contiguous SBUF layout:
    out[p, c*NK*IB + k*IB + m] = xT[k*128 + p, c*IB + m]"""
    a = xT.reshape(NK, 128, NQ, IB)  # [k, p, c, m]
    return np.ascontiguousarray(a.transpose(1, 2, 0, 3).reshape(128, NQ * NK * IB))


def _sbufify_w(w):
    """[1024, E] -> [128, NK*E]: out[p, k*E + e] = w[k*128 + p, e]"""
    e = w.shape[1]
    a = w.reshape(NK, 128, e)
    return np.ascontiguousarray(a.transpose(1, 0, 2).reshape(128, NK * e))


def _prep_inputs(Q, K, V, Wq, bq, Wk, bk, Wv, bv, Wo, bo):
    """Build the 8 per-core input maps (host-side shard + layout)."""
    bf16 = ml_dtypes.bfloat16
    per_batch = []
    for b in range(B):
        xq = _sbufify_x(np.asarray(Q[b]).T).astype(bf16)
        xk = _sbufify_x(np.asarray(K[b]).T).astype(bf16)
        xv = _sbufify_x(np.asarray(V[b]).T).astype(bf16)
        per_batch.append((xq, xk, xv))
    in_maps = []
    for c in range(8):
        b, g = divmod(c, NG)
        xq, xk, xv = per_batch[b]
        gs = slice(g * CG, (g + 1) * CG)
        wq = np.ascontiguousarray(Wq[:, gs]) * 0.125
        wk = np.ascontiguousarray(Wk[:, gs])
        bqs, bks = bq[gs] * 0.125, bk[gs]
        bqk = np.stack(
            [bqs[:128], bks[:128], bqs[128:], bks[128:]], axis=1
        ).astype(np.float32)
        wv = np.zeros((KIN, VW), dtype=np.float32)
        for e in range(HPG):
            wv[:D_MODEL, e * (DK + 1) : e * (DK + 1) + DK] = Wv[
                :, g * CG + e * DK : g * CG + (e + 1) * DK
            ]
            wv[D_MODEL, e * (DK + 1) : e * (DK + 1) + DK] = bv[
                g * CG + e * DK : g * CG + (e + 1) * DK
            ]
            wv[D_MODEL, e * (DK + 1) + DK] = 1.0
        # wv dram layout: [128, NK*VW] weight part; bias row appended at
        # [0:1, NK*VW:NK*VW+VW]
        wv_sb = np.zeros((128, NK * VW + VW), dtype=np.float32)
        wv_sb[:, : NK * VW] = _sbufify_w(wv[:D_MODEL])
        wv_sb[0, NK * VW :] = wv[D_MODEL]
        wo = np.ascontiguousarray(Wo[g * CG : (g + 1) * CG, :])
        in_maps.append(
            {
                "xq": xq,
                "xk": xk,
                "xv": xv,
                "wq": _sbufify_w(wq).astype(bf16),
                "wk": _sbufify_w(wk).astype(bf16),
                "bqk": bqk,
                "wv": wv_sb.astype(bf16),
                "wo": wo.astype(bf16),
            }
        )
    return in_maps


def run(inputs, trace=False):
    if "nc" not in _CACHE:
        _CACHE["nc"] = build_program()
    nc = _CACHE["nc"]
    in_maps = _prep_inputs(**inputs)
    res = run_bass_kernel_spmd(nc, in_maps, core_ids=list(range(8)), trace=trace)
    bo = np.asarray(inputs["bo"], dtype=np.float32)
    outs = []
    for b in range(B):
        acc = res.results[4 * b]["out"].astype(np.float32)
        for g in range(1, NG):
            acc = acc + res.results[4 * b + g]["out"]
        outs.append(acc + bo[None, :])
    return np.stack(outs, axis=0), res


def kernel(**inputs):
    inputs = {k: np.asarray(v) for k, v in inputs.items()}
    out, _ = run(inputs, trace=False)
    return out.astype(np.float32)


# revision 43
# speedup vs baseline: 1.0842x; 1.0842x over previous
"""Multi-head attention (B=2, S=2048, D=1024, H=16) on 8 trn2 NeuronCores.

Sharding: core c handles batch b = c//4 and head-group g = c%4 (4 heads).
Megatron-style: Wq/Wk/Wv column-split, Wo row-split; host sums the 4 partial
outputs per batch and adds bo.

Schedule: the scalar engine (exp over S^2 scores, 128 x [128,1024]
instructions ~142us) is the roofline; everything else is arranged so ACT
never starves:
  - inputs DMA'd in need-order chunks (host pre-laid-out in SBUF shape so
    every DMA is a contiguous 2D copy) on the two HWDGE queues (Sync +
    Scalar; the gpsimd SWDGE queue is slow and its trickle writes contend
    with ACT's SBUF writes); a dummy exp preloads the ACT table during the
    fill
  - per-slot worklists spread qk-projection (2-matmul chunks), v-projection
    (arrival-ordered), AV (lagged, any-j-order accumulation), window finish
    chains, and the output projection across all 8 windows
  - softmax denominators ride the AV matmul as a 65th ones-column (row 64
    of the [65, IB] psum accumulator)
The 1/sqrt(64) score scale is folded into Wq/bq on the host.
"""

import numpy as np
import ml_dtypes
from contextlib import ExitStack

import concourse.bass as bass
import concourse.tile as tile
from concourse import bacc, mybir
from concourse.bass_utils import run_bass_kernel_spmd

F32 = mybir.dt.float32
BF16 = mybir.dt.bfloat16
AF = mybir.ActivationFunctionType

D_MODEL = 1024
NUM_HEADS = 16
DK = 64
B = 2
S = 2048
NG = 4  # head groups = cores per batch
HPG = 4  # heads per group
CG = HPG * DK  # 256 channels per group
KIN = D_MODEL + 1  # wv rows: 1024 + bias row
VW = HPG * (DK + 1)  # 260: per-head [v_h | ones]
IB = 512  # i-block (query) width
NI = S // IB
NJ = S // 128
NK = D_MODEL // 128
NQ = 4  # key quads

_CACHE = {}


def build_program():
    nc = bacc.Bacc("TRN2", target_bir_lowering=False, debug=False, num_devices=8)
    # x tensors are pre-laid-out host-side in SBUF shape [128, 4*NK*IB]:
    # chunk c (i-block / key-quad / token-quarter) at c*NK*IB, k-tile at +k*IB.
    # Every DMA below is a contiguous 2D copy (128 descriptors).
    xq_d = nc.dram_tensor("xq", [128, NI * NK * IB], BF16, kind="ExternalInput")
    xk_d = nc.dram_tensor("xk", [128, NQ * NK * IB], BF16, kind="ExternalInput")
    xv_d = nc.dram_tensor("xv", [128, NQ * NK * IB], BF16, kind="ExternalInput")
    wq_d = nc.dram_tensor("wq", [128, NK * CG], BF16, kind="ExternalInput")
    wk_d = nc.dram_tensor("wk", [128, NK * CG], BF16, kind="ExternalInput")
    bqk_d = nc.dram_tensor("bqk", [128, 4], F32, kind="ExternalInput")
    wv_d = nc.dram_tensor("wv", [128, NK * VW + VW], BF16, kind="ExternalInput")
    wo_d = nc.dram_tensor("wo", [CG, D_MODEL], BF16, kind="ExternalInput")
    out_d = nc.dram_tensor("out", [S, D_MODEL], F32, kind="ExternalOutput")

    windows = [(pr, i) for pr in range(2) for i in range(NI)]

    with tile.TileContext(nc) as tc, ExitStack() as ctx:
        wpool = ctx.enter_context(tc.tile_pool(name="wpool", bufs=1))
        xpool = ctx.enter_context(tc.tile_pool(name="xpool", bufs=1))
        qkvpool = ctx.enter_context(tc.tile_pool(name="qkv", bufs=1))
        attnpool = ctx.enter_context(tc.tile_pool(name="attn", bufs=1))
        spsum = ctx.enter_context(tc.tile_pool(name="spsum", bufs=2, space="PSUM"))
        apsum = ctx.enter_context(tc.tile_pool(name="apsum", bufs=1, space="PSUM"))
        ph1psum = ctx.enter_context(tc.tile_pool(name="ph1psum", bufs=1, space="PSUM"))
        epool = ctx.enter_context(tc.tile_pool(name="epool", bufs=17))
        rpool = ctx.enter_context(tc.tile_pool(name="rpool", bufs=2))
        obounce = ctx.enter_context(tc.tile_pool(name="obounce", bufs=3))

        # ---- SBUF input layouts ----
        # xq: i-block-major; xk: key-quad-major; xv: token-quarter-major
        # (chunk c at c*NK*IB, k-tile k at +k*IB)
        xq_sb = xpool.tile([128, NI * NK * IB], BF16)
        xk_sb = xpool.tile([128, NQ * NK * IB], BF16)
        xv_sb = xpool.tile([128, NQ * NK * IB], BF16)
        wq_sb = wpool.tile([128, NK * CG], BF16)
        wk_sb = wpool.tile([128, NK * CG], BF16)
        wv_sb = wpool.tile([128, NK * VW], BF16)
        bqk_sb = wpool.tile([128, 4], F32)  # cols: [q_cb0, k_cb0, q_cb1, k_cb1]
        wvb = wpool.tile([1, VW], BF16)  # v bias row (incl. ones-col entries)
        xvon = wpool.tile([1, 128], BF16)  # ones row (memset, no DMA)
        wo_sb = [wpool.tile([128, D_MODEL], BF16, name=f"wo{t}") for t in range(2)]
        dum = wpool.tile([1, 8], F32)
        dume = wpool.tile([1, 8], BF16)

        # ---- prologue: prioritized input DMA + act-table preload ----
        nc.gpsimd.memset(xvon[:], 1.0)
        nc.gpsimd.memset(dum[:], 0.0)

        def dma_x(eng, sb, src, c):
            eng.dma_start(
                sb[:, c * NK * IB : (c + 1) * NK * IB],
                src.ap()[:, c * NK * IB : (c + 1) * NK * IB],
            )

        # Two HWDGE queues only (the gpsimd SWDGE queue is far slower and its
        # all-run-long trickle writes contend with ACT's SBUF writes).
        nc.sync.dma_start(bqk_sb[:], bqk_d.ap())
        nc.sync.dma_start(wq_sb[:], wq_d.ap())
        dma_x(nc.sync, xq_sb, xq_d, 0)  # i0
        dma_x(nc.sync, xk_sb, xk_d, 1)  # quad 1
        dma_x(nc.sync, xk_sb, xk_d, 3)  # quad 3
        dma_x(nc.sync, xv_sb, xv_d, 0)
        dma_x(nc.sync, xv_sb, xv_d, 2)
        dma_x(nc.sync, xq_sb, xq_d, 2)  # i2
        dma_x(nc.sync, xq_sb, xq_d, 3)  # i3
        # Scalar queue: all issues happen at t=0 while ACT is otherwise idle
        # (first exp isn't ready until ~15us)
        nc.scalar.dma_start(wk_sb[:], wk_d.ap())
        dma_x(nc.scalar, xk_sb, xk_d, 0)  # quad 0
        dma_x(nc.scalar, xk_sb, xk_d, 2)  # quad 2
        dma_x(nc.scalar, xq_sb, xq_d, 1)  # i1
        nc.scalar.dma_start(wvb[:], wv_d.ap()[0:1, NK * VW : NK * VW + VW])
        nc.scalar.dma_start(wv_sb[:], wv_d.ap()[:, : NK * VW])
        dma_x(nc.scalar, xv_sb, xv_d, 1)
        dma_x(nc.scalar, xv_sb, xv_d, 3)
        for t in range(2):
            nc.scalar.dma_start(wo_sb[t][:], wo_d.ap()[t * 128 : (t + 1) * 128, :])
        nc.scalar.activation(dume[:], dum[:], AF.Exp)  # preload the exp table set

        # ---- persistent compute tiles ----
        qTt = [[qkvpool.tile([128, IB], BF16, name=f"qT{cb}_{i}") for i in range(NI)]
               for cb in range(2)]
        kTt = [[qkvpool.tile([128, IB], BF16, name=f"kT{cb}_{q}") for q in range(NQ)]
               for cb in range(2)]
        vt = [qkvpool.tile([128, VW], BF16, name=f"v{j}") for j in range(NJ)]
        attt = [[attnpool.tile([128, IB], BF16, name=f"att{t}_{i}") for i in range(NI)]
                for t in range(2)]

        # ---- work-unit emitters ----
        qk_state = {}

        def qk_chunk(cb, tsel, i, c):
            """chunk c (0..3) of projection group -> qTt/kTt[cb][i]; 2 mms each."""
            xt, xoff, w_sb, dst, bcol = (
                (xq_sb, i * NK * IB, wq_sb, qTt, 0)
                if tsel == 0
                else (xk_sb, i * NK * IB, wk_sb, kTt, 1)
            )
            key = (cb, tsel, i)
            if c == 0:
                qk_state[key] = ph1psum.tile([128, IB], F32, name="pqk")
            pq = qk_state[key]
            for k in (2 * c, 2 * c + 1):
                nc.tensor.matmul(
                    pq[:],
                    w_sb[:, k * CG + cb * 128 : k * CG + (cb + 1) * 128],
                    xt[:, xoff + k * IB : xoff + (k + 1) * IB],
                    start=(k == 0),
                    stop=(k == NK - 1),
                )
            if c == 3:
                nc.vector.tensor_scalar_add(
                    dst[cb][i][:], pq[:], bqk_sb[:, 2 * cb + bcol : 2 * cb + bcol + 1]
                )
                del qk_state[key]

        vg_done = set()

        def v_group(j):
            """v[j, e] = sum_k xv[k, j] wv[k, e] (+bias/ones via the 1025th row)"""
            vq, jj = j // 4, j % 4
            pv = ph1psum.tile([128, IB], F32, name="p0")[:, :VW]
            for k in range(NK):
                nc.tensor.matmul(
                    pv[:],
                    xv_sb[:, vq * NK * IB + k * IB + jj * 128 :
                          vq * NK * IB + k * IB + (jj + 1) * 128],
                    wv_sb[:, k * VW : (k + 1) * VW],
                    start=(k == 0),
                    stop=False,
                )
            nc.tensor.matmul(pv[:], xvon[:], wvb[:], start=False, stop=True)
            nc.vector.tensor_copy(vt[j][:], pv[:])
            vg_done.add(j)

        # AV: lagged, any-j-order accumulation into [65, IB] psum pair
        pending = []  # (w, j, ee)
        aa_of = {}
        av_count = {}

        def emit_av(w, j, ee):
            pr, i = windows[w]
            if w not in aa_of:
                aa_of[w] = [apsum.tile([DK + 1, IB], F32, name=f"a{u}") for u in range(2)]
                av_count[w] = 0
            first = av_count[w] == 0
            last = av_count[w] == NJ - 1
            for u in range(2):
                h = 2 * pr + u
                nc.tensor.matmul(
                    aa_of[w][u][:],
                    vt[j][:, h * (DK + 1) : (h + 1) * (DK + 1)],
                    ee[:, u * IB : (u + 1) * IB],
                    start=first,
                    stop=last,
                )
            av_count[w] += 1

        def finish_window(w):
            """normalize A^T by the softmax sums (row 64) -> attt[pr][i]"""
            pr, i = windows[w]
            aa = aa_of.pop(w)
            for u in range(2):
                asb = rpool.tile([DK + 1, IB], F32, name="asb")
                nc.vector.tensor_copy(asb[:], aa[u][:])
                r4 = rpool.tile([128, 4], F32, name="r4")
                nc.sync.dma_start(r4[:], asb[DK : DK + 1, :])
                r4b = rpool.tile([128, 4], F32, name="r4b")
                nc.vector.reciprocal(r4b[:], r4[:])
                r0 = rpool.tile([1, IB], F32, name="r0")
                nc.sync.dma_start(r0[:], r4b[:])
                rb = rpool.tile([DK, IB], F32, name="rb")
                nc.gpsimd.partition_broadcast(rb[:], r0[:])
                po = DK * u
                if po == 0:
                    nc.vector.tensor_mul(attt[pr][i][0:DK, :], asb[0:DK, :], rb[:])
                else:
                    nrm = rpool.tile([DK, IB], BF16, name="nrm")
                    nc.vector.tensor_mul(nrm[:], asb[0:DK, :], rb[:])
                    nc.sync.dma_start(attt[pr][i][po : po + DK, :], nrm[:])

        def av_drain(cur_w, cap, target):
            done = 0
            while done < cap and pending:
                pw0 = pending[0][0]
                if pw0 == cur_w and len(pending) <= target:
                    break
                idx = next(
                    (n for n, (pw, pj, _) in enumerate(pending)
                     if pw == pw0 and pj in vg_done),
                    None,
                )
                if idx is None:
                    break
                pw, pj, pee = pending.pop(idx)
                emit_av(pw, pj, pee)
                done += 1
                if av_count[pw] == NJ:
                    finish_window(pw)
                    break  # give the asb copies a slot before the next window's AV

        def force_drain_through(tw):
            """guarantee finish_window(w<=tw) is emitted before returning"""
            while pending and (pending[0][0] <= tw or tw in aa_of):
                pw0 = pending[0][0]
                if pw0 > tw:
                    break
                idx = next(
                    (n for n, (pw, pj, _) in enumerate(pending)
                     if pw == pw0 and pj in vg_done),
                    None,
                )
                if idx is None:
                    break
                pw, pj, pee = pending.pop(idx)
                emit_av(pw, pj, pee)
                if av_count[pw] == NJ:
                    finish_window(pw)

        def ph3_unit(i, ibl):
            """output projection for one 128-token block; per-mh [128,512] store.
            mh0 uses the p0 bank, mh1 the pqk bank (idle once projections are
            done) so the 4 matmuls run back-to-back and copies overlap."""
            force_drain_through(4 + i)
            for mh, pname in ((0, "p0"), (1, "pqk")):
                po_t = ph1psum.tile([128, IB], F32, name=pname)
                for t in range(2):
                    nc.tensor.matmul(
                        po_t[:],
                        attt[t][i][:, ibl * 128 : (ibl + 1) * 128],
                        wo_sb[t][:, mh * IB : (mh + 1) * IB],
                        start=(t == 0),
                        stop=(t == 1),
                    )
                ob = obounce.tile([128, IB], F32, name="ob")
                nc.vector.tensor_copy(ob[:], po_t[:])
                nc.sync.dma_start(
                    out_d.ap()[(i * 4 + ibl) * 128 : (i * 4 + ibl + 1) * 128,
                               mh * IB : (mh + 1) * IB],
                    ob[:],
                )

        # ---- slot schedule ----
        sched = [dict() for _ in range(8)]

        def put(w, j, fn):
            sched[w].setdefault(j, []).append(fn)

        def put_group(w, j0, cb, tsel, i):
            for c in range(4):
                put(w, j0 + c, lambda cb=cb, t=tsel, i=i, c=c: qk_chunk(cb, t, i, c))

        # qk-projection spread (deadlines: qT[pr][i] before window (pr,i);
        # kT[cb][q] before the scores quad that reads it)
        put_group(0, 3, 0, 1, 2)   # KT(0,2) by w0 slot 8
        put_group(0, 8, 0, 1, 3)   # KT(0,3) by w0 slot 12
        put_group(0, 11, 0, 0, 1)  # QT(0,1) by w1
        put_group(1, 12, 0, 0, 2)  # QT(0,2) by w2 (xq-i2 lands mid-w1)
        put_group(2, 2, 0, 0, 3)
        put_group(2, 6, 1, 1, 0)
        put_group(2, 10, 1, 1, 1)
        put_group(3, 1, 1, 1, 2)
        put_group(3, 5, 1, 1, 3)
        put_group(3, 9, 1, 0, 0)
        put_group(4, 1, 1, 0, 1)
        put_group(4, 8, 1, 0, 2)
        put_group(5, 1, 1, 0, 3)
        # v-projection, arrival-ordered (xv quarters interleave both queues)
        for n, (w, j) in enumerate(
            [(0, 14), (0, 15)]
            + [(1, s) for s in range(12)]
            + [(2, 0), (2, 1)]
        ):
            put(w, j, lambda n=n: v_group(n))
        # output projection: i ready after finish(window (1,i)) fires in w5+i
        for i, w in ((0, 5), (1, 6), (2, 7)):
            for n in range(4):
                put(w, 7 + 2 * n, lambda i=i, n=n: ph3_unit(i, n))

        # AV cadence: (cap, target) per window
        av_plan = [(2, 9), (3, 7), (3, 5), (3, 4), (3, 3), (3, 2), (3, 2), (3, 2)]

        # ---- main loop ----
        _sid, _ = nc.enter_named_scope("steady", False)
        # prologue compute (overlaps the DMA fill)
        for c in range(4):
            qk_chunk(0, 0, 0, c)  # QT(0,0)
        for c in range(4):
            qk_chunk(0, 1, 0, c)  # KT(0,0)
        for c in range(4):
            qk_chunk(0, 1, 1, c)  # KT(0,1)

        for w, (pr, i) in enumerate(windows):
            cap, target = av_plan[w]
            for j in range(NJ):
                ss = spsum.tile([128, 2 * IB], F32, name="ss")
                for u in range(2):
                    nc.tensor.matmul(
                        ss[:, u * IB : (u + 1) * IB],
                        kTt[pr][j // 4][u * DK : (u + 1) * DK,
                                        (j % 4) * 128 : (j % 4 + 1) * 128],
                        qTt[pr][i][u * DK : (u + 1) * DK, :],
                        start=True,
                        stop=True,
                        tile_position=(u * DK, 0),
                    )
                ee = epool.tile([128, 2 * IB], BF16, name="ee")
                nc.scalar.activation(ee[:], ss[:], AF.Exp)
                pending.append((w, j, ee))
                for fn in sched[w].get(j, []):
                    fn()
                av_drain(w, cap, target)

        # ---- tail ----
        while pending:
            av_drain(-1, 99, 0)
        for n in range(4):
            ph3_unit(3, n)
        nc.leave_named_scope("steady", _sid, False)

    nc.compile()
    return nc


def _sbufify_x(xT):
    """[1024 d, 2048 s] -> [128, 4*NK*IB] chunk-contiguous SBUF layout:
    out[p, c*NK*IB + k*IB + m] = xT[k*128 + p, c*IB + m]"""
    a = xT.reshape(NK, 128, NQ, IB)  # [k, p, c, m]
    return np.ascontiguousarray(a.transpose(1, 2, 0, 3).reshape(128, NQ * NK * IB))


def _sbufify_w(w):
    """[1024, E] -> [128, NK*E]: out[p, k*E + e] = w[k*128 + p, e]"""
    e = w.shape[1]
    a = w.reshape(NK, 128, e)
    return np.ascontiguousarray(a.transpose(1, 0, 2).reshape(128, NK * e))


def _prep_inputs(Q, K, V, Wq, bq, Wk, bk, Wv, bv, Wo, bo):
    """Build the 8 per-core input maps (host-side shard + layout)."""
    bf16 = ml_dtypes.bfloat16
    per_batch = []
    for b in range(B):
        xq = _sbufify_x(np.asarray(Q[b]).T).astype(bf16)
        xk = _sbufify_x(np.asarray(K[b]).T).astype(bf16)
        xv = _sbufify_x(np.asarray(V[b]).T).astype(bf16)
        per_batch.append((xq, xk, xv))
    in_maps = []
    for c in range(8):
        b, g = divmod(c, NG)
        xq, xk, xv = per_batch[b]
        gs = slice(g * CG, (g + 1) * CG)
        wq = np.ascontiguousarray(Wq[:, gs]) * 0.125
        wk = np.ascontiguousarray(Wk[:, gs])
        bqs, bks = bq[gs] * 0.125, bk[gs]
        bqk = np.stack(
            [bqs[:128], bks[:128], bqs[128:], bks[128:]], axis=1
        ).astype(np.float32)
        wv = np.zeros((KIN, VW), dtype=np.float32)
        for e in range(HPG):
            wv[:D_MODEL, e * (DK + 1) : e * (DK + 1) + DK] = Wv[
                :, g * CG + e * DK : g * CG + (e + 1) * DK
            ]
            wv[D_MODEL, e * (DK + 1) : e * (DK + 1) + DK] = bv[
                g * CG + e * DK : g * CG + (e + 1) * DK
            ]
            wv[D_MODEL, e * (DK + 1) + DK] = 1.0
        # wv dram layout: [128, NK*VW] weight part; bias row appended at
        # [0:1, NK*VW:NK*VW+VW]
        wv_sb = np.zeros((128, NK * VW + VW), dtype=np.float32)
        wv_sb[:, : NK * VW] = _sbufify_w(wv[:D_MODEL])
        wv_sb[0, NK * VW :] = wv[D_MODEL]
        wo = np.ascontiguousarray(Wo[g * CG : (g + 1) * CG, :])
        in_maps.append(
            {
                "xq": xq,
                "xk": xk,
                "xv": xv,
                "wq": _sbufify_w(wq).astype(bf16),
                "wk": _sbufify_w(wk).astype(bf16),
                "bqk": bqk,
                "wv": wv_sb.astype(bf16),
                "wo": wo.astype(bf16),
            }
        )
    return in_maps


def run(inputs, trace=False):
    if "nc" not in _CACHE:
        _CACHE["nc"] = build_program()
    nc = _CACHE["nc"]
    in_maps = _prep_inputs(**inputs)
    res = run_bass_kernel_spmd(nc, in_maps, core_ids=list(range(8)), trace=trace)
    bo = np.asarray(inputs["bo"], dtype=np.float32)
    outs = []
    for b in range(B):
        acc = res.results[4 * b]["out"].astype(np.float32)
        for g in range(1, NG):
            acc = acc + res.results[4 * b + g]["out"]
        outs.append(acc + bo[None, :])
    return np.stack(outs, axis=0), res


def kernel(**inputs):
    inputs = {k: np.asarray(v) for k, v in inputs.items()}
    out, _ = run(inputs, trace=False)
    return out.astype(np.float32)
